# revision 10
# baseline (speedup 1.0000x reference)
"""Trainium2 Bass kernel for 16-head attention (B=4, S=2048, D=1024).

Sharding: 8 cores = 4 batches x 2 head-groups. Core c handles batch c//2,
heads (c%2)*8 .. +8. Each core computes a partial projection output
[S, D]; the host sums the two head-group partials per batch and adds
b_proj. No collectives.

Single software-pipelined emission: the ACT engine (exp over S^2 scores)
is the long pole, so QKV-projection / V / output-projection matmuls are
woven between per-(head-pair, query-chunk) attention blocks as PE filler,
keeping exp saturated from ~15us onward. attn@V matmuls are split into
two K=64 row-strips (rows 0-63 / 64-127 of the PE array) accumulating
into one PSUM tile with a one-j offset so consecutive strip matmuls
co-run, like the row-tiled score pairs. Softmax denominators come from a
ones-augmented 65th V column; 1/denom uses the single-pass
reciprocal_approx_fast custom DVE op and is broadcast across partitions
with a K=1 f32r matmul (1 cycle/row at N=512).
"""

import sys
import os

sys.path.insert(0, "/opt/trn_rl_repo")

import numpy as np
import ml_dtypes

BF = ml_dtypes.bfloat16

DIM = 1024
N_HEADS = 16
HD = 64
B = 4
S = 2048
HPC = 8          # heads per core
GC = HPC * HD    # 512 columns per head-group
N_CORES = 8
SCALE = HD ** -0.5

KD = DIM // 128   # 8 k-tiles over D
NQ = GC // 128    # 4 tiles over the 512 head-group columns (== head-pairs)
NS = S // 512     # 4 seq chunks of 512
ST = S // 128     # 16 seq tiles of 128

_CACHE = {}


def _build_bass():
    import concourse.bass as bass
    import concourse.mybir as mybir
    import concourse.tile as tile
    from concourse import bacc

    f32 = mybir.dt.float32
    f32r = mybir.dt.float32r
    bf16 = mybir.dt.bfloat16
    EXP = mybir.ActivationFunctionType.Exp

    nc = bacc.Bacc("TRN2", target_bir_lowering=False, debug=False,
                   num_devices=N_CORES)

    xT = nc.dram_tensor("xT", [DIM, S], bf16, kind="ExternalInput").ap()
    wq = nc.dram_tensor("wq", [DIM, GC], bf16, kind="ExternalInput").ap()
    wk = nc.dram_tensor("wk", [DIM, GC], bf16, kind="ExternalInput").ap()
    wv = nc.dram_tensor("wv", [DIM, GC], bf16, kind="ExternalInput").ap()
    wp = nc.dram_tensor("wp", [GC, DIM], bf16, kind="ExternalInput").ap()
    # q/k biases pre-broadcast on host: [128, m-tile*1024], each m block
    # holds the per-partition bias value replicated over 2x512 columns
    bqc = nc.dram_tensor("bqc", [128, 4096], f32, kind="ExternalInput").ap()
    bkc = nc.dram_tensor("bkc", [128, 4096], f32, kind="ExternalInput").ap()
    bvb = nc.dram_tensor("bvb", [128, GC], f32, kind="ExternalInput").ap()
    out = nc.dram_tensor("out", [S, DIM], f32, kind="ExternalOutput").ap()

    with tile.TileContext(nc) as tc:
        with tc.tile_pool(name="const", bufs=1) as cp:
            # input DMAs, k-interleaved; xT split in seq-halves so the
            # K[0]/Q[0] units for seq 0:1024 are gated on only 4MB of input
            xTs, wqs, wks, wvs = [], [], [], []
            for k in range(KD):
                t = cp.tile([128, S], bf16, name=f"wxs{k}")
                nc.sync.dma_start(t[:, 0:1024], xT[k * 128:(k + 1) * 128,
                                                   0:1024])
                xTs.append(t)
                for lst, src, nm in ((wks, wk, "k"), (wqs, wq, "q")):
                    t = cp.tile([128, GC], bf16, name=f"w{nm}s{k}")
                    nc.sync.dma_start(t[:], src[k * 128:(k + 1) * 128, :])
                    lst.append(t)
            for k in range(KD):
                nc.sync.dma_start(xTs[k][:, 1024:2048],
                                  xT[k * 128:(k + 1) * 128, 1024:2048])
            bqc_sb = cp.tile([128, 4096], f32, name="bqc_sb")
            nc.sync.dma_start(bqc_sb[:], bqc[:, :])
            bkc_sb = cp.tile([128, 4096], f32, name="bkc_sb")
            nc.sync.dma_start(bkc_sb[:], bkc[:, :])
            bvb_sb = cp.tile([128, GC], f32, name="bvb_sb")
            nc.sync.dma_start(bvb_sb[:], bvb[:, :])
            for k in range(KD):
                t = cp.tile([128, GC], bf16, name=f"wvs{k}")
                nc.sync.dma_start(t[:], wv[k * 128:(k + 1) * 128, :])
                wvs.append(t)
            wps = []
            for k in range(NQ):
                t = cp.tile([128, DIM], bf16, name=f"wps{k}")
                nc.sync.dma_start(t[:], wp[k * 128:(k + 1) * 128, :])
                wps.append(t)
            ones_bf = cp.tile([128, 64], bf16, name="ones_bf")
            nc.any.memset(ones_bf[:], 1.0)

            QT = [cp.tile([128, S], bf16, name=f"QT{m}") for m in range(NQ)]
            KT = [cp.tile([128, S], bf16, name=f"KT{m}") for m in range(NQ)]
            # V tiles: per head 65 cols (64 data + trailing ones column)
            Vt = [cp.tile([128, HPC * 65], bf16, name=f"Vt{s}")
                  for s in range(ST)]
            OT = [cp.tile([128, S], bf16, name=f"OT{m}") for m in range(NQ)]

            for s in range(ST):
                ones_cols = Vt[s][:, :].rearrange(
                    "p (h c) -> p h c", c=65)[:, :, 64:65]
                nc.any.memset(ones_cols, 1.0)

            with tc.tile_pool(name="ss", bufs=1, space="PSUM") as ssp, \
                 tc.tile_pool(name="mi", bufs=1, space="PSUM") as mip, \
                 tc.tile_pool(name="oa", bufs=1, space="PSUM") as oap, \
                 tc.tile_pool(name="pbuf", bufs=4) as pbufp, \
                 tc.tile_pool(name="un", bufs=3) as unp, \
                 tc.tile_pool(name="rr", bufs=2) as rrp, \
                 tc.tile_pool(name="ob", bufs=2) as obp, \
                 tc.tile_pool(name="stg", bufs=3) as stgp:

                def misc_tile(name):
                    return mip.tile([128, 1024], f32, tag="m", bufs=1,
                                    name=name)

                # ---- filler generators (PE work between attention ops) ----
                def gen_qk(dst, ws, bias_sb, m, n2):
                    ps = misc_tile(f"qk{m}{n2}{id(ws) % 97}")
                    for k in range(KD):
                        for h in range(2):
                            nc.tensor.matmul(
                                ps[:, h * 512:(h + 1) * 512],
                                lhsT=ws[k][:, m * 128:(m + 1) * 128],
                                rhs=xTs[k][:, (n2 * 2 + h) * 512:
                                           (n2 * 2 + h + 1) * 512],
                                start=(k == 0), stop=(k == KD - 1))
                            yield
                    nc.vector.tensor_add(
                        dst[m][:, n2 * 1024:(n2 + 1) * 1024], ps[:],
                        bias_sb[:, m * 1024:(m + 1) * 1024])

                def gen_v(s2):
                    ps = misc_tile(f"v{s2}")
                    for k in range(KD):
                        for h in range(2):
                            st = (s2 * 2 + h) * 128
                            nc.tensor.matmul(
                                ps[:, h * 512:(h + 1) * 512],
                                lhsT=xTs[k][:, st:st + 128],
                                rhs=wvs[k][:, :],
                                start=(k == 0), stop=(k == KD - 1))
                            yield
                    for h in range(2):
                        src3 = ps[:, h * 512:(h + 1) * 512].rearrange(
                            "p (g c) -> p g c", c=64)
                        bv3 = bvb_sb[:].rearrange("p (g c) -> p g c", c=64)
                        dst3 = Vt[s2 * 2 + h][:, :].rearrange(
                            "p (g c) -> p g c", c=65)[:, :, 0:64]
                        nc.vector.tensor_add(dst3, src3, bv3)

                def gen_proj(mg):
                    ps = misc_tile(f"pj{mg}")
                    for k in range(NQ):
                        for h in range(2):
                            nc.tensor.matmul(
                                ps[:, h * 512:(h + 1) * 512],
                                lhsT=OT[k][:, mg * 128:(mg + 1) * 128],
                                rhs=wps[k][:, h * 512:(h + 1) * 512],
                                start=(k == 0), stop=(k == NQ - 1))
                            yield
                    ob = obp.tile([128, 1024], f32, tag="ob", name=f"ob{mg}")
                    nc.vector.tensor_copy(ob[:], ps[:])
                    nc.sync.dma_start(out[mg * 128:(mg + 1) * 128, :], ob[:])

                def drive(gens, n):
                    done = 0
                    while gens and done < n:
                        try:
                            next(gens[0])
                            done += 1
                        except StopIteration:
                            gens.pop(0)

                def finish(gens):
                    while gens:
                        try:
                            next(gens[0])
                        except StopIteration:
                            gens.pop(0)

                # ---- normalize chain (deferred into the next block) ----
                def emit_normalize(p):
                    hp, n, u, r = p
                    sq = slice(n * 512, (n + 1) * 512)
                    pb = misc_tile(f"pb{hp}{n}")
                    for half in range(2):
                        nc.tensor.matmul(
                            pb[0:64, half * 512:(half + 1) * 512],
                            lhsT=ones_bf[64:65, 0:64],
                            rhs=r[64:65, half * 512:(half + 1) * 512],
                            start=True, stop=True)
                    nc.vector.tensor_mul(OT[hp][0:64, sq], u[0:64, 0:512],
                                         pb[0:64, 0:512])
                    stB = stgp.tile([64, 512], bf16, tag="st",
                                    name=f"stB{hp}{n}")
                    nc.vector.tensor_mul(stB[:], u[0:64, 512:1024],
                                         pb[0:64, 512:1024])
                    nc.sync.dma_start(OT[hp][64:128, sq], stB[:])

                # ---- attention block ----
                pending = None

                def emit_block(hp, n, fillers, first):
                    nonlocal pending
                    sq = slice(n * 512, (n + 1) * 512)
                    ha = hp * 2
                    oA = oap.tile([128, 512], f32, tag="o", bufs=2,
                                  name=f"oA{hp}{n}")
                    oB = oap.tile([128, 512], f32, tag="o", bufs=2,
                                  name=f"oB{hp}{n}")
                    prev_pT = None
                    for j in range(ST):
                        if first and j % 2 == 0:
                            # V tiles j, j+1 must exist before attnV below
                            finish([fillers.pop(0)])
                        sk = slice(j * 128, (j + 1) * 128)
                        sS = ssp.tile([128, 1024], f32, tag="s", bufs=2,
                                      name=f"sS{hp}{n}{j}")
                        nc.tensor.matmul(
                            sS[:, 0:512], lhsT=KT[hp][0:64, sk],
                            rhs=QT[hp][0:64, sq], start=True, stop=True)
                        nc.tensor.matmul(
                            sS[:, 512:1024], lhsT=KT[hp][64:128, sk],
                            rhs=QT[hp][64:128, sq], start=True, stop=True)
                        pT = pbufp.tile([128, 1024], bf16, tag="p",
                                        name=f"pT{hp}{n}{j}")
                        nc.scalar.activation(pT[:], sS[:], EXP, scale=SCALE)
                        if j == 0 and pending is not None:
                            emit_normalize(pending)
                            pending = None
                        nc.tensor.matmul(
                            oA[0:65, :],
                            lhsT=Vt[j][:, ha * 65:ha * 65 + 65],
                            rhs=pT[:, 0:512],
                            start=(j == 0), stop=(j == ST - 1))
                        nc.tensor.matmul(
                            oB[0:65, :],
                            lhsT=Vt[j][:, ha * 65 + 65:ha * 65 + 130],
                            rhs=pT[:, 512:1024],
                            start=(j == 0), stop=(j == ST - 1))
                        if not first:
                            drive(fillers, 2)
                        prev_pT = pT
                    # evacuate psum accumulators; compute 1/denom right away
                    u = unp.tile([128, 1024], f32, tag="u", name=f"u{hp}{n}")
                    nc.vector.tensor_copy(u[0:65, 0:512], oA[0:65, :])
                    nc.vector.tensor_copy(u[0:65, 512:1024], oB[0:65, :])
                    rb = rrp.tile([128, 1024], bf16, tag="rb",
                                  name=f"rb{hp}{n}")
                    with nc.allow_low_precision(
                            reason="bf16 softmax denom matches bf16 "
                                   "matmul precision"):
                        nc.vector.reciprocal(rb[64:65, 0:1024],
                                             u[64:65, 0:1024])
                    pending = (hp, n, u, rb)
                    finish(fillers)

                # ---- pipeline ----
                prelude = [gen_qk(KT, wks, bkc_sb, 0, 0),
                           gen_qk(KT, wks, bkc_sb, 0, 1),
                           gen_qk(QT, wqs, bqc_sb, 0, 0),
                           gen_qk(QT, wqs, bqc_sb, 0, 1)]
                finish(prelude)

                fillmap = {
                    (0, 0): lambda: [gen_v(s2) for s2 in range(8)],
                    (0, 1): lambda: [gen_qk(KT, wks, bkc_sb, 1, 0),
                                     gen_qk(KT, wks, bkc_sb, 1, 1)],
                    (0, 2): lambda: [gen_qk(QT, wqs, bqc_sb, 1, 0),
                                     gen_qk(QT, wqs, bqc_sb, 1, 1)],
                    (1, 0): lambda: [gen_qk(KT, wks, bkc_sb, 2, 0)],
                    (1, 1): lambda: [gen_qk(KT, wks, bkc_sb, 2, 1)],
                    (1, 2): lambda: [gen_qk(QT, wqs, bqc_sb, 2, 0)],
                    (1, 3): lambda: [gen_qk(QT, wqs, bqc_sb, 2, 1)],
                    (2, 0): lambda: [gen_qk(KT, wks, bkc_sb, 3, 0)],
                    (2, 1): lambda: [gen_qk(KT, wks, bkc_sb, 3, 1)],
                    (2, 2): lambda: [gen_qk(QT, wqs, bqc_sb, 3, 0)],
                    (2, 3): lambda: [gen_qk(QT, wqs, bqc_sb, 3, 1)],
                    (3, 1): lambda: [gen_proj(mg) for mg in range(4)],
                    (3, 2): lambda: [gen_proj(mg) for mg in range(4, 8)],
                    (3, 3): lambda: [gen_proj(mg) for mg in range(8, 12)],
                }
                for hp in range(NQ):
                    for n in range(NS):
                        fillers = fillmap.get((hp, n), lambda: [])()
                        emit_block(hp, n, fillers, first=(hp == 0 and n == 0))
                emit_normalize(pending)
                pending = None
                finish([gen_proj(mg) for mg in range(12, 16)])
    nc.compile()
    return nc


def _get_nc():
    if "nc" not in _CACHE:
        _CACHE["nc"] = _build_bass()
    return _CACHE["nc"]


def _in_maps(x, w_qkv, b_qkv, w_proj, b_proj):
    x = np.asarray(x, np.float32)
    w_qkv = np.asarray(w_qkv, np.float32)
    b_qkv = np.asarray(b_qkv, np.float32)
    w_proj = np.asarray(w_proj, np.float32)

    def bias_bcast(b512):
        col = b512.reshape(4, 128).T[:, :, None]
        return np.ascontiguousarray(
            np.broadcast_to(col, (128, 4, 1024)).reshape(128, 4096))

    maps = []
    for c in range(N_CORES):
        b, g = divmod(c, 2)
        cols = slice(g * GC, (g + 1) * GC)
        wqs = w_qkv[:, 0 * DIM:1 * DIM][:, cols]
        wks = w_qkv[:, 1 * DIM:2 * DIM][:, cols]
        wvs = w_qkv[:, 2 * DIM:3 * DIM][:, cols]
        bqs = b_qkv[0 * DIM:1 * DIM][cols]
        bks = b_qkv[1 * DIM:2 * DIM][cols]
        bvs = b_qkv[2 * DIM:3 * DIM][cols]
        rows = slice(g * GC, (g + 1) * GC)
        maps.append({
            "xT": np.ascontiguousarray(x[b].T).astype(BF),
            "wq": wqs.astype(BF),
            "wk": wks.astype(BF),
            "wv": wvs.astype(BF),
            "wp": w_proj[rows, :].astype(BF),
            "bqc": bias_bcast(bqs),
            "bkc": bias_bcast(bks),
            "bvb": np.broadcast_to(bvs, (128, GC)).copy(),
        })
    return maps


def kernel(x, w_qkv, b_qkv, w_proj, b_proj, _trace=False):
    import time
    from concourse import bass_utils
    nc = _get_nc()
    maps = _in_maps(x, w_qkv, b_qkv, w_proj, b_proj)
    try:
        res = bass_utils.run_bass_kernel_spmd(nc, maps,
                                              core_ids=list(range(N_CORES)),
                                              trace=_trace)
    except Exception:
        # a previously wedged device usually clears after one failed
        # attempt; retry once
        time.sleep(5)
        res = bass_utils.run_bass_kernel_spmd(nc, maps,
                                              core_ids=list(range(N_CORES)),
                                              trace=_trace)
    _CACHE["last_result"] = res
    b_proj = np.asarray(b_proj, np.float32)
    outs = np.empty((B, S, DIM), np.float32)
    for b in range(B):
        outs[b] = (res.results[2 * b]["out"] + res.results[2 * b + 1]["out"]
                   + b_proj)
    return outs


# revision 11
# speedup vs baseline: 1.4213x; 1.4213x over previous
"""Trainium2 Bass kernel for 16-head attention (B=4, S=2048, D=1024).

Sharding: 8 cores = 4 batches x 2 head-groups. Core c handles batch c//2,
heads (c%2)*8 .. +8. Each core computes a partial projection output
[S, D]; the host sums the two head-group partials per batch and adds
b_proj. No collectives.

Single software-pipelined emission: the ACT engine (exp over S^2 scores)
is the long pole, so QKV-projection / V / output-projection matmuls are
woven between per-(head-pair, query-chunk) attention blocks as PE filler,
keeping exp saturated from ~15us onward. attn@V matmuls are split into
two K=64 row-strips (rows 0-63 / 64-127 of the PE array) accumulating
into one PSUM tile with a one-j offset so consecutive strip matmuls
co-run, like the row-tiled score pairs. Softmax denominators come from a
ones-augmented 65th V column; 1/denom uses the single-pass
reciprocal_approx_fast custom DVE op and is broadcast across partitions
with a K=1 f32r matmul (1 cycle/row at N=512).
"""

import sys
import os

sys.path.insert(0, "/opt/trn_rl_repo")

import numpy as np
import ml_dtypes

BF = ml_dtypes.bfloat16

DIM = 1024
N_HEADS = 16
HD = 64
B = 4
S = 2048
HPC = 8          # heads per core
GC = HPC * HD    # 512 columns per head-group
N_CORES = 8
SCALE = HD ** -0.5

KD = DIM // 128   # 8 k-tiles over D
NQ = GC // 128    # 4 tiles over the 512 head-group columns (== head-pairs)
NS = S // 512     # 4 seq chunks of 512
ST = S // 128     # 16 seq tiles of 128

_CACHE = {}


def _build_bass():
    import concourse.bass as bass
    import concourse.mybir as mybir
    import concourse.tile as tile
    from concourse import bacc

    f32 = mybir.dt.float32
    f32r = mybir.dt.float32r
    bf16 = mybir.dt.bfloat16
    EXP = mybir.ActivationFunctionType.Exp

    nc = bacc.Bacc("TRN2", target_bir_lowering=False, debug=False,
                   num_devices=N_CORES)

    xT = nc.dram_tensor("xT", [DIM, S], bf16, kind="ExternalInput").ap()
    wq = nc.dram_tensor("wq", [DIM, GC], bf16, kind="ExternalInput").ap()
    wk = nc.dram_tensor("wk", [DIM, GC], bf16, kind="ExternalInput").ap()
    wv = nc.dram_tensor("wv", [DIM, GC], bf16, kind="ExternalInput").ap()
    wp = nc.dram_tensor("wp", [GC, DIM], bf16, kind="ExternalInput").ap()
    # q/k biases pre-broadcast on host: [128, m-tile*1024], each m block
    # holds the per-partition bias value replicated over 2x512 columns
    bqc = nc.dram_tensor("bqc", [128, 4096], f32, kind="ExternalInput").ap()
    bkc = nc.dram_tensor("bkc", [128, 4096], f32, kind="ExternalInput").ap()
    bvb = nc.dram_tensor("bvb", [128, GC], f32, kind="ExternalInput").ap()
    out = nc.dram_tensor("out", [S, DIM], f32, kind="ExternalOutput").ap()

    with tile.TileContext(nc) as tc:
        with tc.tile_pool(name="const", bufs=1) as cp:
            # input DMAs, k-interleaved; xT split in seq-halves so the
            # K[0]/Q[0] units for seq 0:1024 are gated on only 4MB of input
            xTs, wqs, wks, wvs = [], [], [], []
            for k in range(KD):
                t = cp.tile([128, S], bf16, name=f"wxs{k}")
                nc.sync.dma_start(t[:, 0:1024], xT[k * 128:(k + 1) * 128,
                                                   0:1024])
                xTs.append(t)
                for lst, src, nm in ((wks, wk, "k"), (wqs, wq, "q")):
                    t = cp.tile([128, GC], bf16, name=f"w{nm}s{k}")
                    nc.sync.dma_start(t[:], src[k * 128:(k + 1) * 128, :])
                    lst.append(t)
            for k in range(KD):
                nc.sync.dma_start(xTs[k][:, 1024:2048],
                                  xT[k * 128:(k + 1) * 128, 1024:2048])
            bqc_sb = cp.tile([128, 4096], f32, name="bqc_sb")
            nc.sync.dma_start(bqc_sb[:], bqc[:, :])
            bkc_sb = cp.tile([128, 4096], f32, name="bkc_sb")
            nc.sync.dma_start(bkc_sb[:], bkc[:, :])
            bvb_sb = cp.tile([128, GC], f32, name="bvb_sb")
            nc.sync.dma_start(bvb_sb[:], bvb[:, :])
            for k in range(KD):
                t = cp.tile([128, GC], bf16, name=f"wvs{k}")
                nc.sync.dma_start(t[:], wv[k * 128:(k + 1) * 128, :])
                wvs.append(t)
            wps = []
            for k in range(NQ):
                t = cp.tile([128, DIM], bf16, name=f"wps{k}")
                nc.sync.dma_start(t[:], wp[k * 128:(k + 1) * 128, :])
                wps.append(t)
            ones_bf = cp.tile([128, 64], bf16, name="ones_bf")
            nc.any.memset(ones_bf[:], 1.0)

            QT = [cp.tile([128, S], bf16, name=f"QT{m}") for m in range(NQ)]
            KT = [cp.tile([128, S], bf16, name=f"KT{m}") for m in range(NQ)]
            # V tiles: per head 65 cols (64 data + trailing ones column)
            Vt = [cp.tile([128, HPC * 65], bf16, name=f"Vt{s}")
                  for s in range(ST)]
            OT = [cp.tile([128, S], bf16, name=f"OT{m}") for m in range(NQ)]

            for s in range(ST):
                ones_cols = Vt[s][:, :].rearrange(
                    "p (h c) -> p h c", c=65)[:, :, 64:65]
                nc.any.memset(ones_cols, 1.0)

            with tc.tile_pool(name="ss", bufs=1, space="PSUM") as ssp, \
                 tc.tile_pool(name="mi", bufs=1, space="PSUM") as mip, \
                 tc.tile_pool(name="oa", bufs=1, space="PSUM") as oap, \
                 tc.tile_pool(name="pbuf", bufs=4) as pbufp, \
                 tc.tile_pool(name="un", bufs=3) as unp, \
                 tc.tile_pool(name="rr", bufs=2) as rrp, \
                 tc.tile_pool(name="ob", bufs=2) as obp, \
                 tc.tile_pool(name="stg", bufs=3) as stgp:

                def misc_tile(name):
                    return mip.tile([128, 1024], f32, tag="m", bufs=1,
                                    name=name)

                # ---- filler generators (PE work between attention ops) ----
                def gen_qk(dst, ws, bias_sb, m, n2):
                    ps = misc_tile(f"qk{m}{n2}{id(ws) % 97}")
                    for k in range(KD):
                        for h in range(2):
                            nc.tensor.matmul(
                                ps[:, h * 512:(h + 1) * 512],
                                lhsT=ws[k][:, m * 128:(m + 1) * 128],
                                rhs=xTs[k][:, (n2 * 2 + h) * 512:
                                           (n2 * 2 + h + 1) * 512],
                                start=(k == 0), stop=(k == KD - 1))
                            yield
                    nc.vector.tensor_add(
                        dst[m][:, n2 * 1024:(n2 + 1) * 1024], ps[:],
                        bias_sb[:, m * 1024:(m + 1) * 1024])

                def gen_v(s2):
                    ps = misc_tile(f"v{s2}")
                    for k in range(KD):
                        for h in range(2):
                            st = (s2 * 2 + h) * 128
                            nc.tensor.matmul(
                                ps[:, h * 512:(h + 1) * 512],
                                lhsT=xTs[k][:, st:st + 128],
                                rhs=wvs[k][:, :],
                                start=(k == 0), stop=(k == KD - 1))
                            yield
                    for h in range(2):
                        src3 = ps[:, h * 512:(h + 1) * 512].rearrange(
                            "p (g c) -> p g c", c=64)
                        bv3 = bvb_sb[:].rearrange("p (g c) -> p g c", c=64)
                        dst3 = Vt[s2 * 2 + h][:, :].rearrange(
                            "p (g c) -> p g c", c=65)[:, :, 0:64]
                        nc.vector.tensor_add(dst3, src3, bv3)

                def gen_proj(mg):
                    ps = misc_tile(f"pj{mg}")
                    for k in range(NQ):
                        for h in range(2):
                            nc.tensor.matmul(
                                ps[:, h * 512:(h + 1) * 512],
                                lhsT=OT[k][:, mg * 128:(mg + 1) * 128],
                                rhs=wps[k][:, h * 512:(h + 1) * 512],
                                start=(k == 0), stop=(k == NQ - 1))
                            yield
                    ob = obp.tile([128, 1024], f32, tag="ob", name=f"ob{mg}")
                    nc.vector.tensor_copy(ob[:], ps[:])
                    nc.sync.dma_start(out[mg * 128:(mg + 1) * 128, :], ob[:])

                def drive(gens, n):
                    done = 0
                    while gens and done < n:
                        try:
                            next(gens[0])
                            done += 1
                        except StopIteration:
                            gens.pop(0)

                def finish(gens):
                    while gens:
                        try:
                            next(gens[0])
                        except StopIteration:
                            gens.pop(0)

                # ---- normalize chain (deferred into the next block) ----
                def emit_normalize(p):
                    hp, n, u, r = p
                    sq = slice(n * 512, (n + 1) * 512)
                    pb = misc_tile(f"pb{hp}{n}")
                    for half in range(2):
                        nc.tensor.matmul(
                            pb[0:64, half * 512:(half + 1) * 512],
                            lhsT=ones_bf[64:65, 0:64],
                            rhs=r[64:65, half * 512:(half + 1) * 512],
                            start=True, stop=True)
                    nc.vector.tensor_mul(OT[hp][0:64, sq], u[0:64, 0:512],
                                         pb[0:64, 0:512])
                    stB = stgp.tile([64, 512], bf16, tag="st",
                                    name=f"stB{hp}{n}")
                    nc.vector.tensor_mul(stB[:], u[0:64, 512:1024],
                                         pb[0:64, 512:1024])
                    nc.sync.dma_start(OT[hp][64:128, sq], stB[:])

                # ---- attention block ----
                pending = None

                def emit_block(hp, n, fillers, first):
                    nonlocal pending
                    sq = slice(n * 512, (n + 1) * 512)
                    ha = hp * 2
                    oA = oap.tile([128, 512], f32, tag="o", bufs=2,
                                  name=f"oA{hp}{n}")
                    oB = oap.tile([128, 512], f32, tag="o", bufs=2,
                                  name=f"oB{hp}{n}")
                    prev_pT = None
                    for j in range(ST):
                        if first and j % 2 == 0:
                            # V tiles j, j+1 must exist before attnV below
                            finish([fillers.pop(0)])
                        sk = slice(j * 128, (j + 1) * 128)
                        sS = ssp.tile([128, 1024], f32, tag="s", bufs=2,
                                      name=f"sS{hp}{n}{j}")
                        nc.tensor.matmul(
                            sS[:, 0:512], lhsT=KT[hp][0:64, sk],
                            rhs=QT[hp][0:64, sq], start=True, stop=True)
                        nc.tensor.matmul(
                            sS[:, 512:1024], lhsT=KT[hp][64:128, sk],
                            rhs=QT[hp][64:128, sq], start=True, stop=True)
                        pT = pbufp.tile([128, 1024], bf16, tag="p",
                                        name=f"pT{hp}{n}{j}")
                        nc.scalar.activation(pT[:], sS[:], EXP, scale=SCALE)
                        if j == 0 and pending is not None:
                            emit_normalize(pending)
                            pending = None
                        nc.tensor.matmul(
                            oA[0:65, :],
                            lhsT=Vt[j][:, ha * 65:ha * 65 + 65],
                            rhs=pT[:, 0:512],
                            start=(j == 0), stop=(j == ST - 1))
                        nc.tensor.matmul(
                            oB[0:65, :],
                            lhsT=Vt[j][:, ha * 65 + 65:ha * 65 + 130],
                            rhs=pT[:, 512:1024],
                            start=(j == 0), stop=(j == ST - 1))
                        if not first:
                            drive(fillers, 2)
                        prev_pT = pT
                    # evacuate psum accumulators; compute 1/denom right away
                    u = unp.tile([128, 1024], f32, tag="u", name=f"u{hp}{n}")
                    nc.vector.tensor_copy(u[0:65, 0:512], oA[0:65, :])
                    nc.vector.tensor_copy(u[0:65, 512:1024], oB[0:65, :])
                    # dense-reciprocal trick: the 1024 denominators sit in one
                    # partition row; DMA-reshape them across 128 partitions,
                    # run one cheap [128,8] reciprocal, DMA back into row 64
                    g = rrp.tile([128, 8], f32, tag="g", name=f"g{hp}{n}")
                    nc.sync.dma_start(g[:, :], u[64:65, 0:1024])
                    gb = rrp.tile([128, 8], bf16, tag="gb", name=f"gb{hp}{n}")
                    with nc.allow_low_precision(
                            reason="bf16 softmax denom matches bf16 "
                                   "matmul precision"):
                        nc.vector.reciprocal(gb[:, :], g[:, :])
                    rb = rrp.tile([128, 1024], bf16, tag="rb",
                                  name=f"rb{hp}{n}")
                    nc.sync.dma_start(rb[64:65, 0:1024], gb[:, :])
                    pending = (hp, n, u, rb)
                    finish(fillers)

                # ---- pipeline ----
                prelude = [gen_qk(KT, wks, bkc_sb, 0, 0),
                           gen_qk(KT, wks, bkc_sb, 0, 1),
                           gen_qk(QT, wqs, bqc_sb, 0, 0),
                           gen_qk(QT, wqs, bqc_sb, 0, 1)]
                finish(prelude)

                fillmap = {
                    (0, 0): lambda: [gen_v(s2) for s2 in range(8)],
                    (0, 1): lambda: [gen_qk(KT, wks, bkc_sb, 1, 0),
                                     gen_qk(KT, wks, bkc_sb, 1, 1)],
                    (0, 2): lambda: [gen_qk(QT, wqs, bqc_sb, 1, 0),
                                     gen_qk(QT, wqs, bqc_sb, 1, 1)],
                    (1, 0): lambda: [gen_qk(KT, wks, bkc_sb, 2, 0)],
                    (1, 1): lambda: [gen_qk(KT, wks, bkc_sb, 2, 1)],
                    (1, 2): lambda: [gen_qk(QT, wqs, bqc_sb, 2, 0)],
                    (1, 3): lambda: [gen_qk(QT, wqs, bqc_sb, 2, 1)],
                    (2, 0): lambda: [gen_qk(KT, wks, bkc_sb, 3, 0)],
                    (2, 1): lambda: [gen_qk(KT, wks, bkc_sb, 3, 1)],
                    (2, 2): lambda: [gen_qk(QT, wqs, bqc_sb, 3, 0)],
                    (2, 3): lambda: [gen_qk(QT, wqs, bqc_sb, 3, 1)],
                    (3, 1): lambda: [gen_proj(mg) for mg in range(4)],
                    (3, 2): lambda: [gen_proj(mg) for mg in range(4, 8)],
                    (3, 3): lambda: [gen_proj(mg) for mg in range(8, 12)],
                }
                for hp in range(NQ):
                    for n in range(NS):
                        fillers = fillmap.get((hp, n), lambda: [])()
                        emit_block(hp, n, fillers, first=(hp == 0 and n == 0))
                emit_normalize(pending)
                pending = None
                finish([gen_proj(mg) for mg in range(12, 16)])
    nc.compile()
    return nc


def _get_nc():
    if "nc" not in _CACHE:
        _CACHE["nc"] = _build_bass()
    return _CACHE["nc"]


def _in_maps(x, w_qkv, b_qkv, w_proj, b_proj):
    x = np.asarray(x, np.float32)
    w_qkv = np.asarray(w_qkv, np.float32)
    b_qkv = np.asarray(b_qkv, np.float32)
    w_proj = np.asarray(w_proj, np.float32)

    def bias_bcast(b512):
        col = b512.reshape(4, 128).T[:, :, None]
        return np.ascontiguousarray(
            np.broadcast_to(col, (128, 4, 1024)).reshape(128, 4096))

    maps = []
    for c in range(N_CORES):
        b, g = divmod(c, 2)
        cols = slice(g * GC, (g + 1) * GC)
        wqs = w_qkv[:, 0 * DIM:1 * DIM][:, cols]
        wks = w_qkv[:, 1 * DIM:2 * DIM][:, cols]
        wvs = w_qkv[:, 2 * DIM:3 * DIM][:, cols]
        bqs = b_qkv[0 * DIM:1 * DIM][cols]
        bks = b_qkv[1 * DIM:2 * DIM][cols]
        bvs = b_qkv[2 * DIM:3 * DIM][cols]
        rows = slice(g * GC, (g + 1) * GC)
        maps.append({
            "xT": np.ascontiguousarray(x[b].T).astype(BF),
            "wq": wqs.astype(BF),
            "wk": wks.astype(BF),
            "wv": wvs.astype(BF),
            "wp": w_proj[rows, :].astype(BF),
            "bqc": bias_bcast(bqs),
            "bkc": bias_bcast(bks),
            "bvb": np.broadcast_to(bvs, (128, GC)).copy(),
        })
    return maps


def kernel(x, w_qkv, b_qkv, w_proj, b_proj, _trace=False):
    import time
    from concourse import bass_utils
    nc = _get_nc()
    maps = _in_maps(x, w_qkv, b_qkv, w_proj, b_proj)
    try:
        res = bass_utils.run_bass_kernel_spmd(nc, maps,
                                              core_ids=list(range(N_CORES)),
                                              trace=_trace)
    except Exception:
        # a previously wedged device usually clears after one failed
        # attempt; retry once
        time.sleep(5)
        res = bass_utils.run_bass_kernel_spmd(nc, maps,
                                              core_ids=list(range(N_CORES)),
                                              trace=_trace)
    _CACHE["last_result"] = res
    b_proj = np.asarray(b_proj, np.float32)
    outs = np.empty((B, S, DIM), np.float32)
    for b in range(B):
        outs[b] = (res.results[2 * b]["out"] + res.results[2 * b + 1]["out"]
                   + b_proj)
    return outs
